# revision 4
# baseline (speedup 1.0000x reference)
"""Trainium2 Bass kernel for nn_Block_17540646437178 (dense transformer block).

Sharding: data-parallel over B=16 across 8 cores (2 samples/core, no
collectives). All matmuls in bf16 with f32 PSUM accumulation; layernorm
affines, attention scale and gamma_1/gamma_2 are folded into the weights
on the host.

Attention layout trick: scores are computed TRANSPOSED (k-tokens on
partitions) so the key-padding mask becomes a per-partition activation
bias on the Exp eviction, and V gets an appended ones-column so the
softmax denominator falls out of the attn@V matmul (column 64 of each
head's 65-wide block), landing per-partition for the normalize multiply.
"""

import numpy as np
import ml_dtypes

BF16NP = ml_dtypes.bfloat16

B, N, C, H, D = 16, 616, 768, 12, 64
TXT = 40
DFF = 3072
NCORES = 8
S = B // NCORES          # 2 samples per core
EPS = 1e-5
SCALE = D ** -0.5
KC = C // 128            # 6 k-tiles over C
MQK = (2 * C) // 128     # 12 m-tiles over q+k features
KF = DFF // 128          # 24 k-tiles over dff
NT = 5                   # token tiles per sample (616 = 4*128 + 104)
TOK_TILES = [(0, 128), (128, 128), (256, 128), (384, 128), (512, 104)]
Q_CHUNKS = [(0, 512), (512, 104)]    # 616 free-dim chunks
C_CHUNKS = [(0, 512), (512, 256)]    # 768 free-dim chunks
IMG = N - TXT            # 576
IMGTOK = S * IMG         # 1152 = 9*128
TXTTOK = S * TXT         # 80
IMG_CHUNK = 384          # img token chunk for FFN (3 chunks)
NEG = -30000.0


def _slab_kp(wt):
    """[K, M] (K = KT*128) -> [128, KT, M] slab layout (partition-major)."""
    k, m = wt.shape
    kt = k // 128
    assert kt * 128 == k
    return np.ascontiguousarray(wt.reshape(kt, 128, m).transpose(1, 0, 2))


def _bf(a):
    return np.ascontiguousarray(a.astype(np.float32)).astype(BF16NP)


def _f32(a):
    return np.ascontiguousarray(np.asarray(a, dtype=np.float32))


def _bcast128(v):
    return np.ascontiguousarray(np.broadcast_to(v.astype(np.float32), (128, v.shape[0])))


def _colmajor(v, nt):
    """(nt*128,) -> [128, nt] with column t holding partitions of tile t."""
    return np.ascontiguousarray(v.astype(np.float32).reshape(nt, 128).T)


def host_prep(inputs):
    """Fold affines/scales into weights; build slab/broadcast layouts.

    Returns (shared, per_core) where per_core is a list of dicts.
    """
    inp = {k: _f32(v) if np.asarray(v).dtype != np.int32 else np.asarray(v)
           for k, v in inputs.items()}

    g1, g2 = inp["gamma_1"], inp["gamma_2"]

    # --- attention: fold ln1 affine + SCALE into w_qkv ---
    wqkv = inp["w_qkv"] * inp["ln1_g"][None, :]
    qkv_b = np.concatenate([inp["q_bias"],
                            np.zeros_like(inp["v_bias"]),
                            inp["v_bias"]])
    qkv_b = qkv_b + inp["w_qkv"] @ inp["ln1_b"]
    wqkv[:C] *= SCALE
    qkv_b[:C] *= SCALE

    w_qk = _slab_kp(_bf(wqkv[: 2 * C].T))            # [128, 6, 1536] bf16
    w_v = _slab_kp(_bf(wqkv[2 * C:].T))              # [128, 6, 768] bf16
    qk_bias = _colmajor(qkv_b[: 2 * C], MQK)         # [128, 12] f32
    v_bias = _bcast128(qkv_b[2 * C:])                # [128, 768] f32

    # --- proj: fold gamma_1 ---
    wpj = g1[:, None] * inp["w_proj"]
    w_pj = _slab_kp(_bf(wpj.T))                      # [128, 6, 768] bf16
    b_pj = _bcast128(g1 * inp["b_proj"])             # [128, 768] f32

    # --- FFN branches: fold ln2 affine into fc1, gamma_2 into fc2 ---
    def ffn(w1, b1, w2, b2, lg, lb):
        w1e = w1 * lg[None, :]
        b1e = b1 + w1 @ lb
        w2e = g2[:, None] * w2
        b2e = g2 * b2
        return w1e, b1e, w2e, b2e

    w1t, b1t, w2t, b2t = ffn(inp["fc1t_w"], inp["fc1t_b"], inp["fc2t_w"],
                             inp["fc2t_b"], inp["ln2t_g"], inp["ln2t_b"])
    w1i, b1i, w2i, b2i = ffn(inp["fc1i_w"], inp["fc1i_b"], inp["fc2i_w"],
                             inp["fc2i_b"], inp["ln2i_g"], inp["ln2i_b"])

    # text fc1 weights grouped by M-slab for streaming: [24, 128, 6, 128]
    w1t_T = _bf(w1t.T)                               # [768, 3072]
    w1t_m = np.ascontiguousarray(
        w1t_T.reshape(KC, 128, KF, 128).transpose(2, 1, 0, 3))
    w2t_k = np.ascontiguousarray(_bf(w2t.T).reshape(KF, 128, C))  # [24,128,768]
    w1i_s = _slab_kp(_bf(w1i.T))                     # [128, 6, 3072]
    w2i_s = _slab_kp(_bf(w2i.T))                     # [128, 24, 768]
    b1t_c = _colmajor(b1t, KF)                       # [128, 24]
    b1i_c = _colmajor(b1i, KF)
    b2t_b = _bcast128(b2t)                           # [128, 768]
    b2i_b = _bcast128(b2i)

    # --- rpb transposed + k-padded slabs: [12, 128, 5, 616] bf16 ---
    rpbT = np.transpose(inp["relative_position_bias"], (0, 2, 1))  # [H, k, q]
    rpb_pad = np.zeros((H, NT * 128, N), np.float32)
    rpb_pad[:, :N, :] = rpbT
    rpb_slab = _bf(np.ascontiguousarray(
        rpb_pad.reshape(H, NT, 128, N).transpose(0, 2, 1, 3)))

    shared = dict(w_qk=w_qk, w_v=w_v, qk_bias=qk_bias, v_bias=v_bias,
                  w_pj=w_pj, b_pj=b_pj, rpb=rpb_slab,
                  w1t=w1t_m, b1t=b1t_c, w2t=w2t_k, b2t=b2t_b,
                  w1i=w1i_s, b1i=b1i_c, w2i=w2i_s, b2i=b2i_b)

    # --- per-core: x shard + mask bias ---
    mask = np.asarray(inputs["mask"]).astype(np.float32)   # [B, N] 0/1
    mb_full = (1.0 - mask) * NEG                            # [B, N]
    mb_pad = np.full((B, NT * 128), NEG, np.float32)
    mb_pad[:, :N] = mb_full
    per_core = []
    for c in range(NCORES):
        xs = np.ascontiguousarray(inp["x"][c * S:(c + 1) * S])
        mb = np.ascontiguousarray(
            mb_pad[c * S:(c + 1) * S].reshape(S, NT, 128).transpose(0, 2, 1))
        per_core.append(dict(x=xs, maskb=mb))
    return shared, per_core


def build_program():
    """Build the per-core Bass/Tile program. Returns compiled nc."""
    from contextlib import ExitStack
    import concourse.bass as bass
    import concourse.mybir as mybir
    import concourse.tile as tile
    from concourse import bacc
    from concourse.masks import make_identity

    f32 = mybir.dt.float32
    bf16 = mybir.dt.bfloat16
    Af = mybir.ActivationFunctionType
    Ax = mybir.AxisListType
    Op = mybir.AluOpType

    nc = bacc.Bacc("TRN2", target_bir_lowering=False, debug=False,
                   num_devices=NCORES)

    x_d = nc.declare_dram_parameter("x", [S, N, C], f32, isOutput=False)
    mb_d = nc.declare_dram_parameter("maskb", [S, 128, NT], f32, isOutput=False)
    wqk_d = nc.declare_dram_parameter("w_qk", [128, KC, 2 * C], bf16, isOutput=False)
    wv_d = nc.declare_dram_parameter("w_v", [128, KC, C], bf16, isOutput=False)
    qkb_d = nc.declare_dram_parameter("qk_bias", [128, MQK], f32, isOutput=False)
    vb_d = nc.declare_dram_parameter("v_bias", [128, C], f32, isOutput=False)
    wpj_d = nc.declare_dram_parameter("w_pj", [128, KC, C], bf16, isOutput=False)
    bpj_d = nc.declare_dram_parameter("b_pj", [128, C], f32, isOutput=False)
    rpb_d = nc.declare_dram_parameter("rpb", [H, 128, NT, N], bf16, isOutput=False)
    w1t_d = nc.declare_dram_parameter("w1t", [KF, 128, KC, 128], bf16, isOutput=False)
    b1t_d = nc.declare_dram_parameter("b1t", [128, KF], f32, isOutput=False)
    w2t_d = nc.declare_dram_parameter("w2t", [KF, 128, C], bf16, isOutput=False)
    b2t_d = nc.declare_dram_parameter("b2t", [128, C], f32, isOutput=False)
    w1i_d = nc.declare_dram_parameter("w1i", [128, KC, DFF], bf16, isOutput=False)
    b1i_d = nc.declare_dram_parameter("b1i", [128, KF], f32, isOutput=False)
    w2i_d = nc.declare_dram_parameter("w2i", [128, KF, C], bf16, isOutput=False)
    b2i_d = nc.declare_dram_parameter("b2i", [128, C], f32, isOutput=False)
    out_d = nc.declare_dram_parameter("out", [S, N, C], f32, isOutput=True)

    with tile.TileContext(nc) as tc, ExitStack() as ctx:
        # ---------- pools ----------
        pers = ctx.enter_context(tc.tile_pool(name="pers", bufs=1))
        psum = ctx.enter_context(tc.tile_pool(name="psum", bufs=1, space="PSUM"))

        def ps_tile(name, wide):
            if wide > 256:
                return psum.tile([128, 512], f32, name=name, tag="big", bufs=3)
            return psum.tile([128, 256], f32, name=name, tag="sm", bufs=1)

        # ---------- persistent constants ----------
        ident = pers.tile([128, 128], bf16, name="ident")
        make_identity(nc, ident)
        qkb = pers.tile([128, MQK], f32, name="qkb")
        nc.sync.dma_start(qkb[:], qkb_d[:])
        vb = pers.tile([128, C], f32, name="vb")
        nc.sync.dma_start(vb[:], vb_d[:])
        bpj = pers.tile([128, C], f32, name="bpj")
        nc.sync.dma_start(bpj[:], bpj_d[:])
        mb = pers.tile([128, S, NT], f32, name="mb")
        for s in range(S):
            nc.sync.dma_start(mb[:, s, :], mb_d[s])
        b1t = pers.tile([128, KF], f32, name="b1t")
        nc.sync.dma_start(b1t[:], b1t_d[:])
        b1i = pers.tile([128, KF], f32, name="b1i")
        nc.sync.dma_start(b1i[:], b1i_d[:])
        b2t = pers.tile([128, C], f32, name="b2t")
        nc.sync.dma_start(b2t[:], b2t_d[:])
        b2i = pers.tile([128, C], f32, name="b2i")
        nc.sync.dma_start(b2i[:], b2i_d[:])
        x2rep_img = pers.tile([128, 9, C], f32, name="x2rep_img")
        x2rep_txt = pers.tile([128, C], f32, name="x2rep_txt")
        eps_t = pers.tile([128, 1], f32, name="eps_t")
        nc.vector.memset(eps_t[:], EPS)

        # ---------- helpers ----------
        def layer_norm(pool, src_ap, tp, dst_ap):
            """dst(bf16) = (src - mean)/sqrt(var+EPS); src [tp, C] f32."""
            sm = pool.tile([128, 1], f32, name="ln_sm", tag="ln_sm", bufs=4)
            nc.vector.tensor_reduce(sm[0:tp], src_ap, Ax.X, Op.add)
            nm = pool.tile([128, 1], f32, name="ln_nm", tag="ln_nm", bufs=4)
            nc.scalar.mul(nm[0:tp], sm[0:tp], -1.0 / C)
            xc = pool.tile([128, C], f32, name="ln_xc", tag="ln_xc", bufs=2)
            nc.scalar.add(xc[0:tp], src_ap, nm[0:tp])
            sq = pool.tile([128, C], f32, name="ln_sq", tag="ln_sq", bufs=2)
            ssq = pool.tile([128, 1], f32, name="ln_ssq", tag="ln_ssq", bufs=4)
            nc.scalar.activation(sq[0:tp], xc[0:tp], Af.Square,
                                 accum_out=ssq[0:tp])
            std = pool.tile([128, 1], f32, name="ln_std", tag="ln_std", bufs=4)
            nc.scalar.activation(std[0:tp], ssq[0:tp], Af.Sqrt,
                                 bias=eps_t[0:tp], scale=1.0 / C)
            rstd = pool.tile([128, 1], f32, name="ln_rstd", tag="ln_rstd", bufs=4)
            nc.vector.reciprocal(rstd[0:tp], std[0:tp])
            nc.vector.tensor_scalar_mul(dst_ap, xc[0:tp], rstd[0:tp])

        def transpose_block(src_ap, tp, dst_ap):
            """dst[:, 0:tp] (bf16) = src[0:tp, 0:128].T via PE."""
            ps = psum.tile([128, 128], bf16, name="tps", tag="tp", bufs=2)
            nc.tensor.transpose(ps[:, 0:tp], src_ap, ident[0:tp, 0:tp])
            nc.scalar.copy(dst_ap, ps[:, 0:tp])

        # ================= attention era =================
        with tc.tile_pool(name="era", bufs=1) as era:
            xT = {}
            qkT = {}
            vsb = {}
            osb = {}
            x2 = {}

            with tc.tile_pool(name="wqkv", bufs=1) as wp:
                wqk = wp.tile([128, KC, 2 * C], bf16, name="wqk")
                nc.sync.dma_start(wqk[:], wqk_d[:])
                wv = wp.tile([128, KC, C], bf16, name="wv")
                nc.sync.dma_start(wv[:], wv_d[:])

                # ---- LN1 + transpose to xT ----
                for s in range(S):
                    xT[s] = era.tile([128, KC, N], bf16, name=f"xT{s}",
                                     tag="xT", bufs=2)
                    for (t0, tp) in TOK_TILES:
                        xin = era.tile([128, C], f32, name="xin", tag="xin",
                                       bufs=3)
                        nc.sync.dma_start(xin[0:tp], x_d[s, t0:t0 + tp, :])
                        xh = era.tile([128, C], bf16, name="xh", tag="xh",
                                      bufs=2)
                        layer_norm(era, xin[0:tp], tp, xh[0:tp])
                        for f in range(KC):
                            transpose_block(xh[0:tp, f * 128:(f + 1) * 128],
                                            tp, xT[s][:, f, t0:t0 + tp])

                # ---- QKV projections ----
                for s in range(S):
                    qkT[s] = era.tile([128, MQK, N], bf16, name=f"qkT{s}",
                                      tag="qkT", bufs=2)
                    for m in range(MQK):
                        for (q0, qn) in Q_CHUNKS:
                            ps = ps_tile("ps_qk", qn)
                            for k in range(KC):
                                nc.tensor.matmul(
                                    ps[:, 0:qn],
                                    wqk[:, k, m * 128:(m + 1) * 128],
                                    xT[s][:, k, q0:q0 + qn],
                                    start=(k == 0), stop=(k == KC - 1))
                            nc.vector.tensor_scalar_add(
                                qkT[s][:, m, q0:q0 + qn], ps[:, 0:qn],
                                qkb[:, m:m + 1])
                    vsb[s] = era.tile([128, NT, H * 65], bf16, name=f"v{s}",
                                      tag="v", bufs=2)
                    for ti, (t0, tp) in enumerate(TOK_TILES):
                        for (n0, nn) in C_CHUNKS:
                            ps = ps_tile("ps_v", nn)
                            for k in range(KC):
                                nc.tensor.matmul(
                                    ps[0:tp, 0:nn],
                                    xT[s][:, k, t0:t0 + tp],
                                    wv[:, k, n0:n0 + nn],
                                    start=(k == 0), stop=(k == KC - 1))
                            nh = nn // 64
                            h0 = n0 // 64
                            vview = vsb[s][0:tp, ti, :].rearrange(
                                "p (h e) -> p h e", e=65)[:, h0:h0 + nh, 0:64]
                            nc.vector.tensor_add(
                                vview,
                                ps[0:tp, 0:nn].rearrange("p (h e) -> p h e",
                                                         e=64),
                                vb[0:tp, n0:n0 + nn].rearrange(
                                    "p (h e) -> p h e", e=64))
                        ones = vsb[s][0:tp, ti, :].rearrange(
                            "p (h e) -> p h e", e=65)[:, :, 64:65]
                        nc.vector.memset(ones, 1.0)

            # ---- attention core ----
            for s in range(S):
                osb[s] = era.tile([128, NT, C], bf16, name=f"o{s}", tag="o",
                                  bufs=2)
            with tc.tile_pool(name="attn", bufs=1) as apool:
                for h in range(H):
                    rpb = apool.tile([128, NT, N], bf16, name="rpb", tag="rpb",
                                     bufs=2)
                    nc.sync.dma_start(rpb[:], rpb_d[h])
                    mtile = KC + h // 2
                    qtile = h // 2
                    base = (h % 2) * 64
                    for s in range(S):
                        expT = apool.tile([128, NT, N], bf16, name="expT",
                                          tag="expT", bufs=2)
                        for kt, (k0, tp) in enumerate(TOK_TILES):
                            for (q0, qn) in Q_CHUNKS:
                                ps = ps_tile("ps_sc", qn)
                                nc.tensor.matmul(
                                    ps[0:tp, 0:qn],
                                    qkT[s][base:base + 64, mtile, k0:k0 + tp],
                                    qkT[s][base:base + 64, qtile, q0:q0 + qn],
                                    start=True, stop=True)
                                tmp = era.tile([128, N], f32, name="tmp",
                                               tag="tmp", bufs=3)
                                nc.vector.tensor_add(
                                    tmp[0:tp, 0:qn], ps[0:tp, 0:qn],
                                    rpb[0:tp, kt, q0:q0 + qn])
                                nc.scalar.activation(
                                    expT[0:tp, kt, q0:q0 + qn],
                                    tmp[0:tp, 0:qn], Af.Exp,
                                    bias=mb[0:tp, s, kt:kt + 1])
                        for qt, (qq0, qp) in enumerate(TOK_TILES):
                            ops = psum.tile([128, 65], f32, name="ops",
                                            tag="tiny", bufs=2)
                            for kt, (k0, tp) in enumerate(TOK_TILES):
                                nc.tensor.matmul(
                                    ops[0:qp, :],
                                    expT[0:tp, kt, qq0:qq0 + qp],
                                    vsb[s][0:tp, kt, h * 65:(h + 1) * 65],
                                    start=(kt == 0), stop=(kt == NT - 1))
                            rc = era.tile([128, 1], f32, name="rc", tag="rc",
                                          bufs=4)
                            nc.vector.reciprocal(rc[0:qp], ops[0:qp, 64:65])
                            nc.vector.tensor_scalar_mul(
                                osb[s][0:qp, qt, h * 64:(h + 1) * 64],
                                ops[0:qp, 0:64], rc[0:qp])

            # ---- proj + residual ----
            with tc.tile_pool(name="proj", bufs=1) as pp:
                wpj = pp.tile([128, KC, C], bf16, name="wpj")
                nc.sync.dma_start(wpj[:], wpj_d[:])
                for s in range(S):
                    oT = era.tile([128, KC, N], bf16, name=f"oT{s}", tag="xT",
                                  bufs=2)
                    for ti, (t0, tp) in enumerate(TOK_TILES):
                        for f in range(KC):
                            transpose_block(
                                osb[s][0:tp, ti, f * 128:(f + 1) * 128],
                                tp, oT[:, f, t0:t0 + tp])
                    x2[s] = era.tile([128, NT, C], f32, name=f"x2_{s}",
                                     tag="x2", bufs=2)
                    for ti, (t0, tp) in enumerate(TOK_TILES):
                        xres = pp.tile([128, C], f32, name="xres", tag="xres",
                                       bufs=2)
                        nc.sync.dma_start(xres[0:tp], x_d[s, t0:t0 + tp, :])
                        for (n0, nn) in C_CHUNKS:
                            ps = ps_tile("ps_pj", nn)
                            for k in range(KC):
                                nc.tensor.matmul(
                                    ps[0:tp, 0:nn],
                                    oT[:, k, t0:t0 + tp],
                                    wpj[:, k, n0:n0 + nn],
                                    start=(k == 0), stop=(k == KC - 1))
                            tmp = era.tile([128, N], f32, name="tmp",
                                           tag="tmp", bufs=3)
                            nc.vector.tensor_add(tmp[0:tp, 0:nn],
                                                 ps[0:tp, 0:nn],
                                                 bpj[0:tp, n0:n0 + nn])
                            nc.vector.tensor_add(
                                x2[s][0:tp, ti, n0:n0 + nn],
                                tmp[0:tp, 0:nn], xres[0:tp, n0:n0 + nn])

            # ---- repack x2 -> text [80, C] + img [1152 (9x128), C] ----
            for s in range(S):
                nc.sync.dma_start(x2rep_txt[40 * s:40 * s + 40, :],
                                  x2[s][0:40, 0, :])
                # img rows: seq 40..616 -> global 576*s ..
                g = 576 * s
                for kt, (t0, tp) in enumerate(TOK_TILES):
                    p0 = 40 if kt == 0 else 0
                    length = tp - p0
                    src_off = p0
                    while length > 0:
                        j, dp = g // 128, g % 128
                        piece = min(length, 128 - dp)
                        nc.sync.dma_start(
                            x2rep_img[dp:dp + piece, j, :],
                            x2[s][src_off:src_off + piece, kt, :])
                        g += piece
                        src_off += piece
                        length -= piece

        # ================= FFN era =================
        with tc.tile_pool(name="ffn", bufs=1) as fp:
            # LN2 + transpose
            ztT = fp.tile([128, KC, TXTTOK], bf16, name="ztT")
            xh2 = fp.tile([128, C], bf16, name="xh2", tag="xh2", bufs=2)
            layer_norm(fp, x2rep_txt[0:TXTTOK], TXTTOK, xh2[0:TXTTOK])
            for f in range(KC):
                transpose_block(xh2[0:TXTTOK, f * 128:(f + 1) * 128],
                                TXTTOK, ztT[:, f, 0:TXTTOK])
            ziT = fp.tile([128, KC, IMGTOK], bf16, name="ziT")
            for j in range(9):
                xh2 = fp.tile([128, C], bf16, name="xh2", tag="xh2", bufs=2)
                layer_norm(fp, x2rep_img[:, j, :], 128, xh2[:])
                for f in range(KC):
                    transpose_block(xh2[:, f * 128:(f + 1) * 128], 128,
                                    ziT[:, f, j * 128:(j + 1) * 128])

            # ---- text FFN (streamed weights) ----
            with tc.tile_pool(name="wtxt", bufs=1) as wt:
                hgt = fp.tile([128, KF, TXTTOK], bf16, name="hgt")
                for m in range(KF):
                    w1tm = wt.tile([128, KC, 128], bf16, name="w1tm",
                                   tag="w1tm", bufs=3)
                    nc.sync.dma_start(w1tm[:], w1t_d[m])
                    ps = ps_tile("ps_f1t", 512)
                    for k in range(KC):
                        nc.tensor.matmul(ps[:, 0:TXTTOK], w1tm[:, k, :],
                                         ztT[:, k, 0:TXTTOK],
                                         start=(k == 0), stop=(k == KC - 1))
                    nc.scalar.activation(hgt[:, m, 0:TXTTOK], ps[:, 0:TXTTOK],
                                         Af.Gelu, bias=b1t[:, m:m + 1])
                ps0 = ps_tile("ps_f2t0", 512)
                ps1 = ps_tile("ps_f2t1", 256)
                for k in range(KF):
                    w2tk = wt.tile([128, C], bf16, name="w2tk", tag="w2tk",
                                   bufs=3)
                    nc.sync.dma_start(w2tk[:], w2t_d[k])
                    nc.tensor.matmul(ps0[0:TXTTOK, 0:512], hgt[:, k, 0:TXTTOK],
                                     w2tk[:, 0:512],
                                     start=(k == 0), stop=(k == KF - 1))
                    nc.tensor.matmul(ps1[0:TXTTOK, 0:256], hgt[:, k, 0:TXTTOK],
                                     w2tk[:, 512:768],
                                     start=(k == 0), stop=(k == KF - 1))
                ot = fp.tile([128, C], f32, name="ot", tag="ost", bufs=3)
                for (n0, nn), ps in zip(C_CHUNKS, [ps0, ps1]):
                    tmp = fp.tile([128, 512], f32, name="ftmp", tag="ftmp",
                                  bufs=3)
                    nc.vector.tensor_add(tmp[0:TXTTOK, 0:nn], ps[0:TXTTOK, 0:nn],
                                         b2t[0:TXTTOK, n0:n0 + nn])
                    nc.vector.tensor_add(ot[0:TXTTOK, n0:n0 + nn],
                                         tmp[0:TXTTOK, 0:nn],
                                         x2rep_txt[0:TXTTOK, n0:n0 + nn])
                for s in range(S):
                    nc.sync.dma_start(out_d[s, 0:TXT, :],
                                      ot[40 * s:40 * s + 40, :])

            # ---- img FFN (resident weights, 3 token chunks) ----
            w1i = fp.tile([128, KC, DFF], bf16, name="w1i")
            nc.sync.dma_start(w1i[:], w1i_d[:])
            w2i = fp.tile([128, KF, C], bf16, name="w2i")
            nc.sync.dma_start(w2i[:], w2i_d[:])
            for c in range(3):
                q0 = c * IMG_CHUNK
                hgi = fp.tile([128, KF, IMG_CHUNK], bf16, name="hgi",
                              tag="hgi", bufs=1)
                for m in range(KF):
                    ps = ps_tile("ps_f1i", 512)
                    for k in range(KC):
                        nc.tensor.matmul(ps[:, 0:IMG_CHUNK],
                                         w1i[:, k, m * 128:(m + 1) * 128],
                                         ziT[:, k, q0:q0 + IMG_CHUNK],
                                         start=(k == 0), stop=(k == KC - 1))
                    nc.scalar.activation(hgi[:, m, :], ps[:, 0:IMG_CHUNK],
                                         Af.Gelu, bias=b1i[:, m:m + 1])
                for mt in range(3):
                    j = 3 * c + mt
                    ps0 = ps_tile("ps_f2i0", 512)
                    ps1 = ps_tile("ps_f2i1", 256)
                    for k in range(KF):
                        nc.tensor.matmul(ps0[:, 0:512],
                                         hgi[:, k, mt * 128:(mt + 1) * 128],
                                         w2i[:, k, 0:512],
                                         start=(k == 0), stop=(k == KF - 1))
                        nc.tensor.matmul(ps1[:, 0:256],
                                         hgi[:, k, mt * 128:(mt + 1) * 128],
                                         w2i[:, k, 512:768],
                                         start=(k == 0), stop=(k == KF - 1))
                    ot = fp.tile([128, C], f32, name="ot", tag="ost", bufs=3)
                    for (n0, nn), ps in zip(C_CHUNKS, [ps0, ps1]):
                        tmp = fp.tile([128, 512], f32, name="ftmp", tag="ftmp",
                                      bufs=3)
                        nc.vector.tensor_add(tmp[:, 0:nn], ps[:, 0:nn],
                                             b2i[:, n0:n0 + nn])
                        nc.vector.tensor_add(ot[:, n0:n0 + nn], tmp[:, 0:nn],
                                             x2rep_img[:, j, n0:n0 + nn])
                    # DMA out: global img row g = 128*j -> (b, 40 + g%576)
                    g0 = 128 * j
                    p = 0
                    while p < 128:
                        g = g0 + p
                        b = g // IMG
                        piece = min(128 - p, IMG * (b + 1) - g)
                        seq = TXT + g - b * IMG
                        nc.sync.dma_start(out_d[b, seq:seq + piece, :],
                                          ot[p:p + piece, :])
                        p += piece

    nc.compile()
    return nc


_CACHE = {}


def _get_program():
    if "nc" not in _CACHE:
        _CACHE["nc"] = build_program()
    return _CACHE["nc"]


def run(inputs, trace=False):
    from concourse.bass_utils import run_bass_kernel_spmd
    shared, per_core = host_prep(inputs)
    nc = _get_program()
    in_maps = [{**shared, **pc} for pc in per_core]
    res = run_bass_kernel_spmd(nc, in_maps, core_ids=list(range(NCORES)),
                               trace=trace)
    out = np.concatenate([res.results[i]["out"] for i in range(NCORES)],
                         axis=0).astype(np.float32)
    return out, res


def kernel(**inputs):
    out, _ = run(inputs, trace=False)
    return out
